# revision 1
# baseline (speedup 1.0000x reference)
"""DigitCaps dynamic-routing kernel for 8 Trainium2 NeuronCores.

Problem: x(32,16384,8) f32, W(10,16384,8,16) f32 -> v(32,10,16) f32
  u_hat[b,j,p,o] = sum_d x[b,p,d] W[j,p,d,o]   (never materialized!)
  3 routing iterations (softmax over j, weighted sums over p).

Strategy: shard P=16384 over 8 cores (P_loc=2048). Per routing iteration:
  s_part[b,j,o] = sum_{p,d} (c*x)[b,j,p,d] * W[j,p,d,o]     (PE, K=p 128-chunks)
  AllReduce s (20KB) -> v = squash(s)
  z[b,j,p,d]  = sum_o W[j,p,d,o] v[b,j,o]                   (PE, K=(d,o)=128 with
                                                             block-diagonal v rhs)
  uv[b,j,p]   = sum_d x[b,p,d] z[...]                        (DVE, bf16)
  bb += uv ; c = softmax_j(bb)                               (fp32)
Iteration 1 uses c = 0.1 exactly. Final squash + cross-core s-sum on host.
Matmuls run in float32r (TF32-like 1-pass) via AP bitcast; W is streamed
from HBM each phase; the uv-consume path is bf16 on DVE.

Per-core SBUF layouts (p^ = p % 128 on partitions, t = p//128 in 0..15):
  xt  [128, t16, d8, b32]        ws [128, t16, d8, j10, o16]
  wz  [j10, 128=(d*16+o), t16, p128]
"""
import numpy as np
import ml_dtypes
from functools import lru_cache

import concourse.bacc as bacc
import concourse.mybir as mybir
from concourse import tile
from concourse.bass_utils import run_bass_kernel_spmd

F32 = mybir.dt.float32
F32R = mybir.dt.float32r
BF16 = mybir.dt.bfloat16
AX = mybir.AxisListType
ALU = mybir.AluOpType
ACTF = mybir.ActivationFunctionType

B, J, P, D, O = 32, 10, 16384, 8, 16
NCORES = 8
PL = P // NCORES          # 2048
T = PL // 128             # 16 tiles of 128 p's
TG = 4                    # t-group size in z-phase
JO = J * O                # 160


def _emit(nc, n_cores):
    xt = nc.dram_tensor("xt", [128, T, D, B], F32R, kind="ExternalInput")
    xb = nc.dram_tensor("xb", [128, T, D, B], BF16, kind="ExternalInput")
    ws = nc.dram_tensor("ws", [128, T, D, J, O], F32R, kind="ExternalInput")
    wz = nc.dram_tensor("wz", [J, 128, T, 128], F32R, kind="ExternalInput")
    vz = nc.dram_tensor("vz", [128, J, D * B], F32R, kind="ExternalInput")
    s3p = nc.dram_tensor("s3p", [B, JO], F32, kind="ExternalOutput")

    with tile.TileContext(nc) as tc:
        with (
            tc.tile_pool(name="per", bufs=1) as per,        # persistent
            tc.tile_pool(name="wsst", bufs=3) as wsst,      # ws stream
            tc.tile_pool(name="wzst", bufs=3) as wzst,      # wz stream
            tc.tile_pool(name="yp", bufs=2) as yp,
            tc.tile_pool(name="zc", bufs=2) as zc,          # z consume bufs
            tc.tile_pool(name="small", bufs=2) as small,
            tc.tile_pool(name="sps", bufs=2, space="PSUM") as sps,
            tc.tile_pool(name="zps", bufs=2, space="PSUM") as zps,
            tc.tile_pool(name="dram", bufs=2, space="DRAM") as dramp,
        ):
            # warmup collective first: absorbs ncfw's first-collective
            # barrier (~40us) under the iter-0 compute. Contents junk.
            wu_in = dramp.tile([B, 16], F32)
            wu_out = dramp.tile([B, 16], F32)
            wu_sb = small.tile([B, 16], F32)
            nc.gpsimd.memset(wu_sb[:], 0.0)
            nc.sync.dma_start(wu_in[:], wu_sb[:])
            nc.gpsimd.collective_compute(
                "AllReduce", ALU.add,
                replica_groups=[list(range(n_cores))],
                ins=[wu_in[:].opt()], outs=[wu_out[:].opt()],
            )

            x_sb = per.tile([128, T, D, B], F32R)
            nc.sync.dma_start(x_sb[:], xt[:, :, :, :])
            xb_sb = per.tile([128, T, D, B], BF16)
            nc.sync.dma_start(xb_sb[:], xb[:, :, :, :])
            # block-diagonal v holder: rows (d*16+o), cols per j (d*32+b).
            vblk = per.tile([128, J, D * B], F32R)
            nc.sync.dma_start(vblk[:], vz[:, :, :])   # zeros (memset can't f32r)
            bb = per.tile([128, T, J, B], F32)      # routing logits
            e_sb = per.tile([128, T, J, B], F32)    # exp(bb)
            c_sb = per.tile([128, T, J, B], F32R)    # softmax coeffs
            se = per.tile([128, T, B], F32)         # sum_j exp
            rec = per.tile([128, T, B], F32)        # 1/sum

            for it in range(3):
                # ---------------- s-phase ----------------
                s_ps = sps.tile([B, 256], F32)
                if it > 0:
                    # softmax over j: c = exp(bb) / sum_j exp(bb)
                    nc.scalar.activation(e_sb[:], bb[:], ACTF.Exp)
                    nc.vector.tensor_reduce(
                        se[:, :, :, None],
                        e_sb.rearrange("p t j b -> p t b j"),
                        AX.X, ALU.add,
                    )
                    nc.vector.reciprocal(rec[:], se[:])
                    nc.gpsimd.tensor_mul(
                        c_sb[:], e_sb[:],
                        rec[:, :, None, :].broadcast_to([128, T, J, B]),
                    )
                for t in range(T):
                    wst = wsst.tile([128, D, J, O], F32R)
                    nc.sync.dma_start(wst[:], ws[:, t, :, :, :])
                    if it == 0:
                        # c == 0.1 exactly: lhsT = x, scale folded into copy.
                        # N padded 160->256 (reads run into the next d's
                        # region; junk lands in psum cols 160..255, ignored)
                        # to hit fp32r's 1-cycle/row regime; the last chunk
                        # can't overrun the tile so it stays N=160.
                        for d in range(D):
                            rhs = wst.rearrange("p d j o -> p (d j o)")
                            if d == D - 1:  # next-d overrun not possible
                                rhs = rhs[:, d * JO:(d + 1) * JO]
                            else:
                                rhs = rhs[:, d * JO:d * JO + 256]
                            nc.tensor.matmul(
                                s_ps[:, 0:rhs.shape[-1]],
                                x_sb[:, t, d, :],
                                rhs,
                                start=(t == 0 and d == 0),
                                stop=(t == T - 1 and d == D - 1),
                            )
                    else:
                        y_t = yp.tile([128, J, D, B], F32R)
                        # y = c * x, broadcast ops run at 1x -> split the
                        # work between DVE and GpSimd by t parity
                        eng = nc.vector if t % 2 == 0 else nc.gpsimd
                        eng.tensor_mul(
                            y_t[:],
                            c_sb[:, t, :, None, :].broadcast_to([128, J, D, B]),
                            x_sb[:, t, None, :, :].broadcast_to([128, J, D, B]),
                        )
                        for j in range(J):
                            for d in range(D):
                                # single accumulation group per psum bank
                                nc.tensor.matmul(
                                    s_ps[:, j * O:(j + 1) * O],
                                    y_t[:, j, d, :],
                                    wst[:, d, j, :],
                                    start=(t == 0 and j == 0 and d == 0),
                                    stop=(t == T - 1 and j == J - 1 and d == D - 1),
                                )
                s_sb = small.tile([B, JO], F32)
                nc.scalar.activation(s_sb[:], s_ps[:, 0:JO], ACTF.Copy,
                                     scale=0.1 if it == 0 else 1.0)
                if it == 2:
                    nc.sync.dma_start(s3p[:, :], s_sb[:])
                    break

                # ---------------- AllReduce s ----------------
                cc_in = dramp.tile([B, JO], F32)
                cc_out = dramp.tile([B, JO], F32)
                nc.sync.dma_start(cc_in[:], s_sb[:])
                nc.gpsimd.collective_compute(
                    "AllReduce", ALU.add,
                    replica_groups=[list(range(n_cores))],
                    ins=[cc_in[:].opt()], outs=[cc_out[:].opt()],
                )
                s_f = small.tile([B, JO], F32)
                nc.sync.dma_start(s_f[:], cc_out[:])

                # ---------------- squash -> v ----------------
                t2 = small.tile([B, JO], F32)
                nc.vector.tensor_mul(t2[:], s_f[:], s_f[:])
                sq = small.tile([B, J], F32)
                nc.vector.tensor_reduce(
                    sq[:, :, None], t2.rearrange("b (j o) -> b j o", j=J),
                    AX.X, ALU.add)
                r_ = small.tile([B, J], F32)
                nc.scalar.activation(r_[:], sq[:], ACTF.Sqrt)
                den = small.tile([B, J], F32)
                # den = (sq + 1) * r
                nc.vector.scalar_tensor_tensor(
                    den[:], sq[:], 1.0, r_[:], ALU.add, ALU.mult)
                rc2 = small.tile([B, J], F32)
                nc.vector.reciprocal(rc2[:], den[:])
                f_ = small.tile([B, J], F32)
                nc.vector.tensor_mul(f_[:], sq[:], rc2[:])
                v_sb = small.tile([B, J, O], F32R)
                nc.vector.tensor_mul(
                    v_sb[:], s_f.rearrange("b (j o) -> b j o", j=J),
                    f_[:, :, None].broadcast_to([B, J, O]))
                # bounce v through DRAM, then scatter transposed copies into
                # the block-diagonal slots (DMA is exempt from the 32-aligned
                # partition-start rule engine ops have)
                v_dr = dramp.tile([B, J, O], F32R)
                nc.sync.dma_start(v_dr[:], v_sb[:])
                for j in range(J):
                    for d in range(D):
                        nc.sync.dma_start(
                            vblk[d * O:(d + 1) * O, j, d * B:(d + 1) * B],
                            v_dr[:, j, :].rearrange("b o -> o b"))

                # ---------------- z / uv phase ----------------
                for j in range(J):
                    wzs = wzst.tile([128, T, 128], F32R)
                    nc.sync.dma_start(wzs[:], wz[j, :, :, :])
                    for tg in range(T // TG):
                        z_ps = zps.tile([128, TG, D * B], F32)
                        for t4 in range(TG):
                            # two 1KB outputs share each 2KB psum bank ->
                            # pair them into one group per bank
                            nc.tensor.matmul(
                                z_ps[:, t4, :], wzs[:, tg * TG + t4, :],
                                vblk[:, j, :],
                                start=(t4 % 2 == 0), stop=(t4 % 2 == 1))
                        ztmp = zc.tile([128, TG * D * B], BF16)
                        nc.scalar.copy(
                            ztmp[:], z_ps.rearrange("p t db -> p (t db)"))
                        tmp2 = zc.tile([128, TG * D * B], BF16)
                        nc.vector.tensor_mul(
                            tmp2[:], ztmp[:],
                            xb_sb[:, tg * TG:(tg + 1) * TG, :, :]
                            .rearrange("p t d b -> p (t d b)"))
                        t2v = tmp2.rearrange("p (t d b) -> p t d b", t=TG, d=D)
                        u1 = zc.tile([128, TG, 4, B], BF16)
                        nc.vector.tensor_add(
                            u1[:], t2v[:, :, 0:4, :], t2v[:, :, 4:8, :])
                        u2 = zc.tile([128, TG, 2, B], BF16)
                        nc.vector.tensor_add(
                            u2[:], u1[:, :, 0:2, :], u1[:, :, 2:4, :])
                        bb_sl = bb[:, tg * TG:(tg + 1) * TG, j, :]
                        if it == 0:
                            nc.vector.tensor_add(
                                bb_sl, u2[:, :, 0, :], u2[:, :, 1, :])
                        else:
                            uv = zc.tile([128, TG, B], F32)
                            nc.vector.tensor_add(
                                uv[:], u2[:, :, 0, :], u2[:, :, 1, :])
                            nc.vector.tensor_add(bb_sl, bb_sl, uv[:])
    return nc


@lru_cache(maxsize=2)
def _build(n_cores):
    nc = bacc.Bacc("TRN2", target_bir_lowering=False, debug=False,
                   num_devices=n_cores)
    _emit(nc, n_cores)
    nc.compile()
    return nc


def _prep_inputs(x, W):
    """Host-side shard + relayout. Returns list of per-core input dicts."""
    x = np.asarray(x, dtype=np.float32)
    W = np.asarray(W, dtype=np.float32)
    in_maps = []
    for c in range(NCORES):
        xc = x[:, c * PL:(c + 1) * PL, :]              # (B, PL, D)
        Wc = W[:, c * PL:(c + 1) * PL, :, :]           # (J, PL, D, O)
        xr = np.ascontiguousarray(
            xc.reshape(B, T, 128, D).transpose(2, 1, 3, 0))        # [128,T,D,B]
        wsr = np.ascontiguousarray(
            Wc.reshape(J, T, 128, D, O).transpose(2, 1, 3, 0, 4))  # [128,T,D,J,O]
        wzr = np.ascontiguousarray(
            Wc.reshape(J, T, 128, D, O).transpose(0, 3, 4, 1, 2)   # j,d,o,t,p
            .reshape(J, 128, T, 128))                              # [J,(d,o),T,p]
        in_maps.append({"xt": xr, "xb": xr.astype(ml_dtypes.bfloat16),
                        "ws": wsr, "wz": wzr,
                        "vz": np.zeros((128, J, D * B), np.float32)})
    return in_maps


def _squash_np(s):
    sq = np.sum(s * s, axis=-1, keepdims=True)
    return s * (sq / ((1.0 + sq) * np.sqrt(sq)))


def kernel(x, W):
    nc = _build(NCORES)
    in_maps = _prep_inputs(x, W)
    res = run_bass_kernel_spmd(nc, in_maps, list(range(NCORES)))
    s3 = np.zeros((B, JO), np.float64)
    for r in res.results:
        s3 += r["s3p"].astype(np.float64)
    v = _squash_np(s3.reshape(B, J, O))
    return v.astype(np.float32)



# revision 9
# speedup vs baseline: 1.3554x; 1.3554x over previous
"""DigitCaps dynamic-routing kernel for 8 Trainium2 NeuronCores (v2).

Problem: x(32,16384,8) f32, W(10,16384,8,16) f32 -> v(32,10,16) f32
  u_hat[b,j,p,o] = sum_d x[b,p,d] W[j,p,d,o]   (never materialized)
  3 routing iterations (softmax over j, weighted sums over p).

Shard P=16384 over 8 cores (P_loc=2048 = 16 tiles of 128). All weights
bf16, resident in SBUF (no per-iteration W streaming). Per routing step:

  s-phase (PE): s[b,j,o] = sum_{p,d} y * W with y = e * x~ (bf16).
    it0: c==0.1 -> single psum group, lhsT=x [p^,32], rhs=W [p^,160].
    it>0: j-quad packing: lhsT = y[p^,(j4,b)=128], rhs = W[p^,(j4,o)=64],
    out[(j4,b),(j4,o)] - only the j-diagonal [32b,16o] blocks are used;
    3 quad groups (j 0-3, 4-7, 8-9) accumulate over all (t,d) in psum.
    Extraction: 10 cross-partition-base scalar copies psum->sbuf.
  AllReduce s (20KB) -> squash -> v (bf16).
  v transpose via PE (2 transposes) -> DRAM [160,32] -> 8 diagonal DMAs
    build the block-diag vblk[(d,o), j, (d,b)] moving operand.
  z-phase (PE): z[p^,(d,b)] = wz[(do),p^]^T @ vblk (N=256 per (j,t)).
  consume (DVE/Scalar/GpSimd): uv = sum_d x*z, bb += uv. Chunks of
    (j, 4 t's): either DVE mult direct from psum (1x), or scalar drains
    psum->sbuf bf16 then DVE/GpSimd mult at 2x; bf16 add-tree over d.
  softmax: e = exp(bb) (scalar), se = sum_j e, rec = 1/se (bf16),
    x~ = rec * x, y = e * x~  (c = e*rec never materialized).

Final iteration outputs the per-core partial s3p[b, j*16+o]; the host
sums partials in f64 and applies the last squash.
"""
import numpy as np
import ml_dtypes
from functools import lru_cache

import concourse.bacc as bacc
import concourse.mybir as mybir
from concourse import tile
from concourse.bass_utils import run_bass_kernel_spmd

F32 = mybir.dt.float32
BF16 = mybir.dt.bfloat16
AX = mybir.AxisListType
ALU = mybir.AluOpType
ACTF = mybir.ActivationFunctionType

B, J, P, D, O = 32, 10, 16384, 8, 16
NCORES = 8
PL = P // NCORES          # 2048
T = PL // 128             # 16 tiles of 128 p's
TG = 4                    # t-group size for z/consume chunks
NTG = T // TG             # 4
JO = J * O                # 160
QUADS = ((0, 4), (4, 4), (8, 2))   # (j0, nj) quad groups for s-phase

# consume-path split per j (tuned for engine balance; see module docstring)
DIRECT_J = {0, 1, 5, 6}    # DVE mult straight from psum (1x)
SCGP_J = {4, 9}            # scalar drain + gpsimd mult
# remaining j: scalar drain + DVE mult (2x)
YGP_T = {2, 7, 12}         # y-mult tiles computed on gpsimd


def _emit(nc, n_cores):
    xb_d = nc.dram_tensor("xb", [128, T, D, B], BF16, kind="ExternalInput")
    ws_d = nc.dram_tensor("ws", [128, T, D, JO], BF16, kind="ExternalInput")
    wz_d = nc.dram_tensor("wz", [128, J, T, 128], BF16, kind="ExternalInput")
    id_d = nc.dram_tensor("ident", [32, 32], BF16, kind="ExternalInput")
    s3p = nc.dram_tensor("s3p", [B, JO], F32, kind="ExternalOutput")

    with tile.TileContext(nc) as tc:
        with (
            tc.tile_pool(name="per", bufs=1) as per,
            tc.tile_pool(name="yp", bufs=3) as yp,
            tc.tile_pool(name="hap", bufs=4) as hap,
            tc.tile_pool(name="zdp", bufs=3) as zdp,
            tc.tile_pool(name="u1p", bufs=3) as u1p,
            tc.tile_pool(name="u2p", bufs=3) as u2p,
            tc.tile_pool(name="u3p", bufs=3) as u3p,
            tc.tile_pool(name="small", bufs=4) as small,
            tc.tile_pool(name="sps", bufs=1, space="PSUM") as sps,
            tc.tile_pool(name="vtp", bufs=1, space="PSUM") as vtp,
            tc.tile_pool(name="zps", bufs=2, space="PSUM") as zps,
            tc.tile_pool(name="dram", bufs=4, space="DRAM") as dramp,
        ):
            # warmup collective: absorbs ncfw's first-collective barrier
            # (~40us) under the initial DMA loads + it0 compute.
            wu_in = dramp.tile([B, 16], F32)
            wu_out = dramp.tile([B, 16], F32)
            wu_sb = small.tile([B, 16], F32)
            nc.vector.memset(wu_sb[:], 0.0)
            nc.sync.dma_start(wu_in[:], wu_sb[:])
            nc.gpsimd.collective_compute(
                "AllReduce", ALU.add,
                replica_groups=[list(range(n_cores))],
                ins=[wu_in[:].opt()], outs=[wu_out[:].opt()],
            )

            xb = per.tile([128, T, D, B], BF16)
            nc.sync.dma_start(xb[:], xb_d[:, :, :, :])
            ws = per.tile([128, T, D, JO], BF16)
            for t in range(T):
                nc.sync.dma_start(ws[:, t, :, :], ws_d[:, t, :, :])
            wz = per.tile([128, J, T, 128], BF16)
            nc.sync.dma_start(wz[:], wz_d[:, :, :, :])
            ident = per.tile([32, 32], BF16)
            nc.sync.dma_start(ident[:], id_d[:, :])

            bb = per.tile([128, T, J, B], F32)      # routing logits
            e_sb = per.tile([128, T, J, B], BF16)   # exp(bb)
            se = per.tile([128, T, B], F32)         # sum_j exp
            rec = per.tile([128, T, B], BF16)       # 1/sum
            xt = per.tile([128, T, D, B], BF16)     # x~ = rec * x
            vblk = per.tile([128, J, D * B], BF16)  # block-diag v
            nc.vector.memset(vblk[:], 0.0)

            # ---------------- it0 s-phase: c == 0.1 ----------------
            s_ps = sps.tile([32, JO], F32, name="sq0")
            for t in range(T):
                for d in range(D):
                    nc.tensor.matmul(
                        s_ps[:, :], xb[:, t, d, :], ws[:, t, d, :],
                        start=(t == 0 and d == 0),
                        stop=(t == T - 1 and d == D - 1))
            sx = small.tile([B, JO], F32)
            nc.scalar.activation(sx[:], s_ps[:, :], ACTF.Copy, scale=0.1)

            def quad_psums():
                return [sps.tile([128, nj * O], F32, name=f"sq{q}")
                        for q, (_, nj) in enumerate(QUADS)]

            def s_extract(qps, dst):
                for j in range(J):
                    q, jr = (0, j) if j < 4 else ((1, j - 4) if j < 8
                                                  else (2, j - 8))
                    nc.scalar.copy(
                        dst[0:B, j * O:(j + 1) * O],
                        qps[q][32 * jr:32 * jr + 32, O * jr:O * jr + O])

            for k in range(2):      # routing steps that need v (it0, it1)
                # -------- AllReduce s --------
                cc_in = dramp.tile([B, JO], F32)
                cc_out = dramp.tile([B, JO], F32)
                nc.sync.dma_start(cc_in[:], sx[:])
                nc.gpsimd.collective_compute(
                    "AllReduce", ALU.add,
                    replica_groups=[list(range(n_cores))],
                    ins=[cc_in[:].opt()], outs=[cc_out[:].opt()],
                )
                s_f = small.tile([B, JO], F32)
                nc.sync.dma_start(s_f[:], cc_out[:])

                # -------- squash -> v (bf16) --------
                t2 = small.tile([B, JO], F32)
                nc.vector.tensor_mul(t2[:], s_f[:], s_f[:])
                sq = small.tile([B, J], F32)
                nc.vector.tensor_reduce(
                    sq[:, :, None], t2.rearrange("b (j o) -> b j o", j=J),
                    AX.X, ALU.add)
                r_ = small.tile([B, J], F32)
                nc.scalar.activation(r_[:], sq[:], ACTF.Sqrt)
                den = small.tile([B, J], F32)
                nc.vector.scalar_tensor_tensor(
                    den[:], sq[:], 1.0, r_[:], ALU.add, ALU.mult)
                rc2 = small.tile([B, J], F32)
                nc.vector.reciprocal(rc2[:], den[:])
                f_ = small.tile([B, J], F32)
                nc.vector.tensor_mul(f_[:], sq[:], rc2[:])
                v_sb = small.tile([B, JO], BF16)
                nc.vector.tensor_mul(
                    v_sb.rearrange("b (j o) -> b j o", j=J),
                    s_f.rearrange("b (j o) -> b j o", j=J),
                    f_[:, :, None].broadcast_to([B, J, O]))

                # -------- v -> vT (PE transpose) -> DRAM -> vblk --------
                vt1_ps = vtp.tile([128, 32], BF16, name="vt_ps")
                nc.tensor.transpose(vt1_ps[:], v_sb[:, 0:128], ident[:])
                vt1 = small.tile([128, 32], BF16)
                nc.scalar.copy(vt1[:], vt1_ps[:])
                vt2_ps = vtp.tile([32, 32], BF16, name="vt_ps")
                nc.tensor.transpose(vt2_ps[:], v_sb[:, 128:160], ident[:])
                vt2 = small.tile([32, 32], BF16)
                nc.scalar.copy(vt2[:], vt2_ps[:])
                vt_dr = dramp.tile([JO, B], BF16)
                nc.sync.dma_start(vt_dr[0:128, :], vt1[:])
                nc.sync.dma_start(vt_dr[128:160, :], vt2[:])
                vt_v = vt_dr.rearrange("(j o) b -> o j b", j=J)
                for d in range(D):
                    nc.sync.dma_start(
                        vblk[d * O:(d + 1) * O, :, d * B:(d + 1) * B],
                        vt_v[:, :, :])

                # -------- z-phase + consume + (softmax/y/s of k+1) ------
                qps = quad_psums()
                for tg in range(NTG):
                    t0 = tg * TG
                    for j in range(J):
                        z_ps = zps.tile([128, TG, D * B], F32)
                        for t4 in range(TG):
                            nc.tensor.matmul(
                                z_ps[:, t4, :], wz[:, j, t0 + t4, :],
                                vblk[:, j, :],
                                start=(t4 % 2 == 0), stop=(t4 % 2 == 1))
                        xs = xb[:, t0:t0 + TG, :, :].rearrange(
                            "p t d b -> p (t d b)")
                        ha = hap.tile([128, TG * D * B], BF16)
                        zv = z_ps.rearrange("p t db -> p (t db)")
                        if j in DIRECT_J:
                            nc.vector.tensor_mul(ha[:], zv, xs)
                        else:
                            zd = zdp.tile([128, TG * D * B], BF16)
                            nc.scalar.copy(zd[:], zv)
                            eng = nc.gpsimd if j in SCGP_J else nc.vector
                            eng.tensor_mul(ha[:], zd[:], xs)
                        hv = ha.rearrange("p (t d b) -> p t d b", t=TG, d=D)
                        u1 = u1p.tile([128, TG, 4, B], BF16)
                        ueng = nc.gpsimd if j in SCGP_J else nc.vector
                        ueng.tensor_add(
                            u1[:], hv[:, :, 0:4, :], hv[:, :, 4:8, :])
                        u2 = u2p.tile([128, TG, 2, B], BF16)
                        nc.vector.tensor_add(
                            u2[:], u1[:, :, 0:2, :], u1[:, :, 2:4, :])
                        bb_sl = bb[:, t0:t0 + TG, j, :]
                        if k == 0:
                            nc.vector.tensor_add(
                                bb_sl, u2[:, :, 0, :], u2[:, :, 1, :])
                        else:
                            u3 = u3p.tile([128, TG, B], BF16)
                            nc.vector.tensor_add(
                                u3[:], u2[:, :, 0, :], u2[:, :, 1, :])
                            nc.vector.tensor_add(bb_sl, bb_sl, u3[:])

                    # ---- softmax for this t-group (bb complete) ----
                    nc.scalar.activation(
                        e_sb[:, t0:t0 + TG, :, :], bb[:, t0:t0 + TG, :, :],
                        ACTF.Exp)
                    nc.vector.tensor_reduce(
                        se[:, t0:t0 + TG, :, None],
                        e_sb[:, t0:t0 + TG, :, :].rearrange(
                            "p t j b -> p t b j"),
                        AX.X, ALU.add)
                    with nc.allow_low_precision(
                            reason="1/se as bf16 feeds bf16 x~; validated"):
                        nc.vector.reciprocal(
                            rec[:, t0:t0 + TG, :], se[:, t0:t0 + TG, :])
                    nc.vector.tensor_mul(
                        xt[:, t0:t0 + TG, :, :],
                        xb[:, t0:t0 + TG, :, :],
                        rec[:, t0:t0 + TG, None, :].broadcast_to(
                            [128, TG, D, B]))
                    # ---- y + s-matmuls of step k+1 for this t-group ----
                    for t in range(t0, t0 + TG):
                        y_t = yp.tile([128, D, J, B], BF16)
                        yeng = nc.gpsimd if t in YGP_T else nc.vector
                        yeng.tensor_mul(
                            y_t[:],
                            xt[:, t, :, None, :].broadcast_to([128, D, J, B]),
                            e_sb[:, t, None, :, :].broadcast_to(
                                [128, D, J, B]))
                        for d in range(D):
                            for q, (j0, nj) in enumerate(QUADS):
                                nc.tensor.matmul(
                                    qps[q][0:nj * 32, :],
                                    y_t[:, d, j0:j0 + nj, :],
                                    ws[:, t, d, j0 * O:(j0 + nj) * O],
                                    start=(t == t0 and tg == 0 and d == 0),
                                    stop=(t == t0 + TG - 1 and tg == NTG - 1
                                          and d == D - 1))
                sx = small.tile([B, JO], F32)
                s_extract(qps, sx)

            nc.sync.dma_start(s3p[:, :], sx[:])
    return nc


@lru_cache(maxsize=2)
def _build(n_cores):
    nc = bacc.Bacc("TRN2", target_bir_lowering=False, debug=False,
                   num_devices=n_cores)
    _emit(nc, n_cores)
    nc.compile()
    return nc


def _prep_inputs(x, W):
    """Host-side shard + relayout. Returns list of per-core input dicts."""
    x = np.asarray(x, dtype=np.float32)
    W = np.asarray(W, dtype=np.float32)
    ident = np.eye(32, dtype=ml_dtypes.bfloat16)
    in_maps = []
    for c in range(NCORES):
        xc = x[:, c * PL:(c + 1) * PL, :]              # (B, PL, D)
        Wc = W[:, c * PL:(c + 1) * PL, :, :]           # (J, PL, D, O)
        xr = np.ascontiguousarray(
            xc.reshape(B, T, 128, D).transpose(2, 1, 3, 0))        # [128,T,D,B]
        wsr = np.ascontiguousarray(
            Wc.reshape(J, T, 128, D, O).transpose(2, 1, 3, 0, 4)
            .reshape(128, T, D, JO))                               # [128,T,D,JO]
        wzr = np.ascontiguousarray(
            Wc.reshape(J, T, 128, D, O).transpose(3, 4, 0, 1, 2)
            .reshape(128, J, T, 128))                              # [(d,o),J,T,p]
        in_maps.append({
            "xb": xr.astype(ml_dtypes.bfloat16),
            "ws": wsr.astype(ml_dtypes.bfloat16),
            "wz": wzr.astype(ml_dtypes.bfloat16),
            "ident": ident,
        })
    return in_maps


def _squash_np(s):
    sq = np.sum(s * s, axis=-1, keepdims=True)
    return s * (sq / ((1.0 + sq) * np.sqrt(sq)))


def kernel(x, W):
    nc = _build(NCORES)
    in_maps = _prep_inputs(x, W)
    res = run_bass_kernel_spmd(nc, in_maps, list(range(NCORES)))
    s3 = np.zeros((B, JO), np.float64)
    for r in res.results:
        s3 += r["s3p"].astype(np.float64)
    v = _squash_np(s3.reshape(B, J, O))
    return v.astype(np.float32)
